# revision 50
# baseline (speedup 1.0000x reference)
"""Trainium2 Bass kernel for Bahdanau-style attention (nn_Attention).

  s_t   = concat(h_dec, c_dec)                      [B, D]
  EF    = enc @ Wh^T                                [B, S, D]
  df    = s_t @ Ws^T + bs                           [B, D]
  score = tanh(EF + df) @ v                         [B, S]
  w     = softmax(score) * mask, renormalized       [B, S]
  ctx   = w @ enc                                   [B, D]
  returns (ctx, w)

Sharding: data-parallel over batch B=32 across 8 cores (4 examples/core),
weights replicated. No collectives.

Per-core device plan (all heavy compute in bf16, fp32 accumulation):
  1. enc (fp32) is cast-loaded to bf16 via SWDGE DMA, stored to a DRAM
     scratch, and transpose-loaded back with the DMA xbar so the
     contraction dim d lands on partitions (encT[d, s]).
  2. EF^T[d_out, s] tiles = sum_k WhT[k,m].T @ encT[k, s] on TensorE.
     tanh(x + df) fused on ScalarE with per-partition bias, output bf16.
  3. score = v . T via M=1 matmuls accumulated over the 8 d_out tiles.
  4. softmax without max-subtraction (|score| <= sum|v| ~ 32, exp is safe
     in fp32 and mathematically identical to the max-subtracted form).
     mask apply + sum in one fused tensor_tensor_reduce.
  5. context = sum_s w[s] * enc[s, :] via M=1 matmuls over natural-layout
     bf16 enc tiles kept from step 1; normalized by 1/sum at the end.
"""

import os
import sys

for _p in ("/opt/trn_rl_repo", "/root/.axon_site/_ro/trn_rl_repo"):
    if os.path.isdir(_p) and _p not in sys.path:
        sys.path.insert(0, _p)

import ml_dtypes
import numpy as np

import concourse.bacc as bacc
import concourse.mybir as mybir
import concourse.tile as tile
from concourse.bass_utils import run_bass_kernel_spmd

BF16 = mybir.dt.bfloat16
F32 = mybir.dt.float32
NPBF16 = ml_dtypes.bfloat16

H = 512
D = 2 * H          # 1024
B = 32
S = 2048
NCORES = 8
BPC = B // NCORES  # 4 examples per core
P = 128            # partitions
KT = D // P        # 8 contraction tiles
MT = D // P        # 8 d_out tiles
ST = S // P        # 16 s tiles
CH = 512           # s-chunk width (max fp32 PSUM free dim)
NCHUNK = S // CH   # 4

AF = mybir.ActivationFunctionType
ALU = mybir.AluOpType

_CACHE = {}
LAST_EXEC_NS = None
LAST_RESULTS = None


def _build_nc():
    nc = bacc.Bacc("TRN2", target_bir_lowering=False, debug=False,
                   enable_asserts=False)

    enc_d = nc.dram_tensor("enc", [BPC, S, D], F32, kind="ExternalInput").ap()
    wht_d = nc.dram_tensor("wht", [MT, KT, P, P], BF16,
                           kind="ExternalInput").ap()
    wst_d = nc.dram_tensor("wst", [KT, P, D], BF16, kind="ExternalInput").ap()
    stt_d = nc.dram_tensor("stt", [KT, P, BPC], BF16, kind="ExternalInput").ap()
    vv_d = nc.dram_tensor("vv", [P, KT], BF16, kind="ExternalInput").ap()
    bsr_d = nc.dram_tensor("bsr", [P, KT], F32, kind="ExternalInput").ap()
    mask_d = nc.dram_tensor("mask", [BPC, S], F32, kind="ExternalInput").ap()
    id_d = nc.dram_tensor("ident128", [P, P], BF16, kind="ExternalInput").ap()
    octx_d = nc.dram_tensor("out_ctx", [BPC, D], F32, kind="ExternalOutput").ap()
    oatt_d = nc.dram_tensor("out_att", [BPC, S], F32, kind="ExternalOutput").ap()

    with tile.TileContext(nc) as tc:
        with (
            tc.tile_pool(name="const", bufs=1) as const_pool,
            tc.tile_pool(name="nat", bufs=8) as nat_pool,
            tc.tile_pool(name="tp", bufs=8) as tp_pool,
            tc.tile_pool(name="tt", bufs=4) as t_pool,
            tc.tile_pool(name="rows", bufs=2) as rows_pool,
            tc.tile_pool(name="smalls", bufs=2) as small_pool,
            tc.tile_pool(name="psum_ef", bufs=3, space="PSUM") as ef_psum,
            tc.tile_pool(name="psum_sc", bufs=2, space="PSUM") as sc_psum,
            tc.tile_pool(name="psum_cx", bufs=1, space="PSUM") as cx_psum,
            tc.tile_pool(name="psum_pt", bufs=2, space="PSUM") as pt_psum,
        ):
            # ---------------- one-time loads (sync HWDGE ring; the big
            # cast-loads are on the gpsimd SWDGE ring). Order matters:
            # small consts first so df + the first transposes can start
            # while the bigger weight tiles stream in. ------------------
            stt_sb = const_pool.tile([P, KT, BPC], BF16)
            nc.sync.dma_start(stt_sb[:], stt_d.rearrange("k p j -> p k j"))
            vv_sb = const_pool.tile([P, KT], BF16)
            nc.sync.dma_start(vv_sb[:], vv_d[:])
            bs_sb = const_pool.tile([P, KT], F32)
            nc.sync.dma_start(bs_sb[:], bsr_d[:])
            id_sb = const_pool.tile([P, P], BF16)
            nc.sync.dma_start(id_sb[:], id_d[:])
            ident = const_pool.tile([1, 1], F32)
            nc.vector.memset(ident[:], 1.0)

            # ---------------- decoder features df ----------------
            # dfb[d_out partition, m, b] = (Ws @ s_t^T)[d_out, b] + bs[d_out]
            # Ws^T streamed in halves through the encT slots.
            dfb = const_pool.tile([P, MT, BPC], F32)
            for half in range(2):
                wst_sb = tp_pool.tile([P, KT, D // 2], BF16, tag="encT",
                                      name=f"wst_sb{half}")
                nc.sync.dma_start(
                    wst_sb[:],
                    wst_d[:, :, half * (D // 2):(half + 1) * (D // 2)]
                    .rearrange("k p e -> p k e"))
                for mm in range(MT // 2):
                    m = half * (MT // 2) + mm
                    ps_df = pt_psum.tile([P, BPC], F32, tag="pt",
                                         name=f"ps_df{m}")
                    for k in range(KT):
                        nc.tensor.matmul(
                            ps_df[:],
                            wst_sb[:, k, mm * P:(mm + 1) * P],
                            stt_sb[:, k, :],
                            start=(k == 0),
                            stop=(k == KT - 1),
                        )
                    nc.vector.tensor_scalar_add(
                        dfb[:, m, :], ps_df[:], bs_sb[:, m:m + 1])

            # Wh^T streamed per m-tile (the first EF chunk only needs m=0
            # to begin; later tiles arrive while earlier ones compute)
            wht_sb = const_pool.tile([P, MT, KT, P], BF16)
            for m in range(MT):
                nc.sync.dma_start(wht_sb[:, m, :, :],
                                  wht_d[m].rearrange("k p j -> p k j"))

            # ---------------- per-example stream ----------------
            stop_stage = os.environ.get("BASSK_STOP", "")
            n_ex = int(os.environ.get("BASSK_NB", BPC))
            for b in range(n_ex):
                # per-quarter chain: cast-load -> spill -> transpose-load.
                # Separate tiles per quarter so the scheduler can overlap
                # quarter q+1's DMA chain with quarter q's matmuls.
                enc_nat_qs = []
                encT_qs = []
                for q in range(4):
                    # 1) cast-load enc (fp32 -> bf16), natural [s, d]
                    enc_nat_q = nat_pool.tile([P, 4, D], BF16, tag="nat",
                                              name=f"enc_nat_{b}_{q}")
                    enc_nat_qs.append(enc_nat_q)
                    nc.gpsimd.dma_start(
                        enc_nat_q[:],
                        enc_d[b, 512 * q:512 * (q + 1), :].rearrange(
                            "(t p) d -> p t d", p=P),
                    )
                    # 2) TensorE transpose -> encT[d partition, s-chunk]
                    # (PE-side transposes keep the DMA rings free and the
                    # PE clock warm; identity is the stationary operand)
                    encT_q = tp_pool.tile([P, KT, CH], BF16, tag="encT",
                                          name=f"encT_{b}_{q}")
                    encT_qs.append(encT_q)
                    for m in range(KT):
                        ps_t = pt_psum.tile([P, CH], BF16, tag="pt",
                                            name=f"ps_t_{b}_{q}_{m}")
                        for t in range(4):
                            nc.tensor.transpose(
                                ps_t[:, t * P:(t + 1) * P],
                                enc_nat_q[:, t, m * P:(m + 1) * P],
                                id_sb[:])
                        nc.vector.tensor_copy(encT_q[:, m, :], ps_t[:])
                # mask row (exactly representable in bf16 for 0/1 masks)
                mask_t = rows_pool.tile([1, S], BF16, tag="mask")
                nc.gpsimd.dma_start(mask_t[:], mask_d[b:b + 1, :])

                if stop_stage == "dma":
                    # keep the DMA chain alive via a dummy output write
                    for q in range(4):
                        nc.gpsimd.dma_start(
                            oatt_d[b:b + 1, q * CH:(q + 1) * CH],
                            encT_qs[q][0:1, 0, :])
                    continue
                # 4) EF^T tiles + fused tanh + score matvec
                score_t = rows_pool.tile([1, S], F32, tag="score")
                for c in range(NCHUNK):
                    ps_sc = sc_psum.tile([1, CH], F32, tag="sc")
                    for m in range(MT):
                        ps_ef = ef_psum.tile([P, CH], F32, tag="ef")
                        for k in range(KT):
                            nc.tensor.matmul(
                                ps_ef[:],
                                wht_sb[:, m, k, :],
                                encT_qs[c][:, k, :],
                                start=(k == 0),
                                stop=(k == KT - 1),
                            )
                        t_t = t_pool.tile([P, CH], BF16, tag="tt")
                        nc.scalar.activation(
                            t_t[:], ps_ef[:], AF.Tanh,
                            bias=dfb[:, m, b:b + 1])
                        if stop_stage == "ef":
                            if m == 0:
                                nc.gpsimd.dma_start(
                                    oatt_d[b:b + 1, c * CH:(c + 1) * CH],
                                    t_t[0:1, :])
                            continue
                        nc.tensor.matmul(
                            ps_sc[:],
                            vv_sb[:, m:m + 1],
                            t_t[:],
                            start=(m == 0),
                            stop=(m == MT - 1),
                        )
                    if stop_stage == "ef":
                        continue
                    nc.vector.tensor_copy(
                        score_t[0:1, c * CH:(c + 1) * CH], ps_sc[:])
                if stop_stage == "ef":
                    continue
                if stop_stage == "score":
                    nc.sync.dma_start(oatt_d[b:b + 1, :], score_t[:])
                    continue

                # 5) masked softmax (no max-subtraction; |score| <= 32)
                e_t = rows_pool.tile([1, S], BF16, tag="e")
                nc.scalar.activation(e_t[:], score_t[:], AF.Exp)
                if stop_stage == "exp":
                    nc.gpsimd.dma_start(oatt_d[b:b + 1, :], e_t[:])
                    continue
                em_t = rows_pool.tile([1, S], F32, tag="em", bufs=1)
                sum_t = small_pool.tile([1, 1], F32, tag="sum")
                nc.vector.tensor_mul(em_t[:], e_t[:], mask_t[:])
                nc.vector.reduce_sum(sum_t[:], em_t[:],
                                     axis=mybir.AxisListType.X)
                if stop_stage == "ttr":
                    nc.sync.dma_start(oatt_d[b:b + 1, 0:1], sum_t[:])
                    nc.sync.dma_start(oatt_d[b:b + 1, 4:4 + S - 4],
                                      em_t[0:1, 0:S - 4])
                    continue
                recip_t = small_pool.tile([1, 1], F32, tag="recip")
                nc.vector.reciprocal(recip_t[:], sum_t[:])

                # attention-weight output row
                att_t = rows_pool.tile([1, S], BF16, tag="e", name="att_t")
                nc.vector.tensor_scalar_mul(att_t[:], em_t[:],
                                            recip_t[0:1, 0:1])
                nc.gpsimd.dma_start(oatt_d[b:b + 1, :], att_t[:])
                if stop_stage == "soft":
                    continue

                # 6) transpose unnormalized weights to partitions
                ps_wt = sc_psum.tile([P, ST], F32, tag="sc", name=f"ps_wt{b}")
                for j in range(ST):
                    nc.tensor.transpose(
                        ps_wt[:, j:j + 1],
                        em_t[0:1, j * P:(j + 1) * P],
                        ident[:])
                w_bf = small_pool.tile([P, ST], BF16, tag="wbf")
                nc.vector.tensor_copy(w_bf[:], ps_wt[:])
                if stop_stage == "wt":
                    nc.gpsimd.dma_start(octx_d[b:b + 1, 0:ST],
                                        w_bf[0:1, :])
                    continue

                # 7) context matvec over natural-layout enc
                ctx_t = rows_pool.tile([1, D], F32, tag="ctx", bufs=1)
                for hh in range(2):
                    ps_cx = cx_psum.tile([1, CH], F32, tag="cx")
                    for st in range(ST):
                        nc.tensor.matmul(
                            ps_cx[:],
                            w_bf[:, st:st + 1],
                            enc_nat_qs[st // 4][:, st % 4,
                                                hh * CH:(hh + 1) * CH],
                            start=(st == 0),
                            stop=(st == ST - 1),
                        )
                    nc.scalar.mul(ctx_t[0:1, hh * CH:(hh + 1) * CH],
                                  ps_cx[:], recip_t[0:1, 0:1])
                nc.scalar.dma_start(octx_d[b:b + 1, :], ctx_t[:])

    nc.compile()
    return nc


def _get_nc():
    if "nc" not in _CACHE:
        _CACHE["nc"] = _build_nc()
    return _CACHE["nc"]


def kernel(h_dec, c_dec, encoder_output, x_padding_masks, Wh, Ws, bs, v,
           _trace=False):
    global LAST_EXEC_NS, LAST_RESULTS
    h = np.asarray(h_dec, dtype=np.float32)
    c = np.asarray(c_dec, dtype=np.float32)
    enc = np.asarray(encoder_output, dtype=np.float32)
    mask = np.asarray(x_padding_masks, dtype=np.float32)
    Wh = np.asarray(Wh, dtype=np.float32)
    Ws = np.asarray(Ws, dtype=np.float32)
    bs = np.asarray(bs, dtype=np.float32)
    v = np.asarray(v, dtype=np.float32)

    # host-side prep of the small parameters only
    # wht[m, k, p, j] = Wh^T[k*128+p, m*128+j] (m-major tile order)
    wht = np.ascontiguousarray(
        Wh.T.reshape(KT, P, MT, P).transpose(2, 0, 1, 3)).astype(NPBF16)
    wst = np.ascontiguousarray(Ws.T).reshape(KT, P, D).astype(NPBF16)
    s_t = np.concatenate([h[0], c[0]], axis=1)          # [B, D]
    stt_full = np.ascontiguousarray(s_t.T)              # [D, B]
    vv = np.ascontiguousarray(v.reshape(KT, P).T).astype(NPBF16)
    bsr = np.ascontiguousarray(bs.reshape(KT, P).T).astype(np.float32)
    ident128 = np.eye(P, dtype=NPBF16)

    in_maps = []
    for i in range(NCORES):
        b0 = i * BPC
        stt_i = np.ascontiguousarray(
            stt_full[:, b0:b0 + BPC]).reshape(KT, P, BPC).astype(NPBF16)
        in_maps.append({
            "enc": np.ascontiguousarray(enc[b0:b0 + BPC]),
            "mask": np.ascontiguousarray(mask[b0:b0 + BPC]),
            "wht": wht,
            "wst": wst,
            "stt": stt_i,
            "vv": vv,
            "bsr": bsr,
            "ident128": ident128,
        })

    nc = _get_nc()
    res = run_bass_kernel_spmd(nc, in_maps, core_ids=list(range(NCORES)),
                               trace=_trace)
    LAST_EXEC_NS = res.exec_time_ns
    LAST_RESULTS = res
    ctx = np.concatenate([r["out_ctx"] for r in res.results], axis=0)
    att = np.concatenate([r["out_att"] for r in res.results], axis=0)
    return ctx, att


# revision 52
# speedup vs baseline: 1.0231x; 1.0231x over previous
"""Trainium2 Bass kernel for Bahdanau-style attention (nn_Attention).

  s_t   = concat(h_dec, c_dec)                      [B, D]
  EF    = enc @ Wh^T                                [B, S, D]
  df    = s_t @ Ws^T + bs                           [B, D]
  score = tanh(EF + df) @ v                         [B, S]
  w     = softmax(score) * mask, renormalized       [B, S]
  ctx   = w @ enc                                   [B, D]
  returns (ctx, w)

Sharding: data-parallel over batch B=32 across 8 cores (4 examples/core),
weights replicated. No collectives.

Per-core device plan (all heavy compute in bf16, fp32 accumulation):
  1. enc (fp32) is cast-loaded to bf16 via SWDGE DMA, stored to a DRAM
     scratch, and transpose-loaded back with the DMA xbar so the
     contraction dim d lands on partitions (encT[d, s]).
  2. EF^T[d_out, s] tiles = sum_k WhT[k,m].T @ encT[k, s] on TensorE.
     tanh(x + df) fused on ScalarE with per-partition bias, output bf16.
  3. score = v . T via M=1 matmuls accumulated over the 8 d_out tiles.
  4. softmax without max-subtraction (|score| <= sum|v| ~ 32, exp is safe
     in fp32 and mathematically identical to the max-subtracted form).
     mask apply + sum in one fused tensor_tensor_reduce.
  5. context = sum_s w[s] * enc[s, :] via M=1 matmuls over natural-layout
     bf16 enc tiles kept from step 1; normalized by 1/sum at the end.
"""

import os
import sys

for _p in ("/opt/trn_rl_repo", "/root/.axon_site/_ro/trn_rl_repo"):
    if os.path.isdir(_p) and _p not in sys.path:
        sys.path.insert(0, _p)

import ml_dtypes
import numpy as np

import concourse.bacc as bacc
import concourse.mybir as mybir
import concourse.tile as tile
from concourse.bass_utils import run_bass_kernel_spmd

BF16 = mybir.dt.bfloat16
F32 = mybir.dt.float32
NPBF16 = ml_dtypes.bfloat16

H = 512
D = 2 * H          # 1024
B = 32
S = 2048
NCORES = 8
BPC = B // NCORES  # 4 examples per core
P = 128            # partitions
KT = D // P        # 8 contraction tiles
MT = D // P        # 8 d_out tiles
ST = S // P        # 16 s tiles
CH = 512           # s-chunk width (max fp32 PSUM free dim)
NCHUNK = S // CH   # 4

AF = mybir.ActivationFunctionType
ALU = mybir.AluOpType

_CACHE = {}
LAST_EXEC_NS = None
LAST_RESULTS = None


def _build_nc():
    nc = bacc.Bacc("TRN2", target_bir_lowering=False, debug=False,
                   enable_asserts=False)

    enc_d = nc.dram_tensor("enc", [BPC, S, D], F32, kind="ExternalInput").ap()
    wht_d = nc.dram_tensor("wht", [MT, KT, P, P], BF16,
                           kind="ExternalInput").ap()
    wst_d = nc.dram_tensor("wst", [KT, P, D], BF16, kind="ExternalInput").ap()
    stt_d = nc.dram_tensor("stt", [KT, P, BPC], BF16, kind="ExternalInput").ap()
    vv_d = nc.dram_tensor("vv", [P, KT], BF16, kind="ExternalInput").ap()
    bsr_d = nc.dram_tensor("bsr", [P, KT], F32, kind="ExternalInput").ap()
    mask_d = nc.dram_tensor("mask", [BPC, S], F32, kind="ExternalInput").ap()
    id_d = nc.dram_tensor("ident128", [P, P], BF16, kind="ExternalInput").ap()
    octx_d = nc.dram_tensor("out_ctx", [BPC, D], F32, kind="ExternalOutput").ap()
    oatt_d = nc.dram_tensor("out_att", [BPC, S], F32, kind="ExternalOutput").ap()

    with tile.TileContext(nc) as tc:
        with (
            tc.tile_pool(name="const", bufs=1) as const_pool,
            tc.tile_pool(name="nat", bufs=8) as nat_pool,
            tc.tile_pool(name="tp", bufs=8) as tp_pool,
            tc.tile_pool(name="tt", bufs=3) as t_pool,
            tc.tile_pool(name="rows", bufs=2) as rows_pool,
            tc.tile_pool(name="smalls", bufs=2) as small_pool,
            tc.tile_pool(name="psum_ef", bufs=2, space="PSUM") as ef_psum,
            tc.tile_pool(name="psum_sc", bufs=2, space="PSUM") as sc_psum,
            tc.tile_pool(name="psum_cx", bufs=2, space="PSUM") as cx_psum,
            tc.tile_pool(name="psum_pt", bufs=2, space="PSUM") as pt_psum,
        ):
            # ---------------- one-time loads (sync HWDGE ring; the big
            # cast-loads are on the gpsimd SWDGE ring). Order matters:
            # small consts first so df + the first transposes can start
            # while the bigger weight tiles stream in. ------------------
            stt_sb = const_pool.tile([P, KT, BPC], BF16)
            nc.sync.dma_start(stt_sb[:], stt_d.rearrange("k p j -> p k j"))
            vv_sb = const_pool.tile([P, KT], BF16)
            nc.sync.dma_start(vv_sb[:], vv_d[:])
            bs_sb = const_pool.tile([P, KT], F32)
            nc.sync.dma_start(bs_sb[:], bsr_d[:])
            id_sb = const_pool.tile([P, P], BF16)
            nc.sync.dma_start(id_sb[:], id_d[:])
            ident = const_pool.tile([1, 1], F32)
            nc.vector.memset(ident[:], 1.0)

            # ---------------- decoder features df ----------------
            # dfb[d_out partition, m, b] = (Ws @ s_t^T)[d_out, b] + bs[d_out]
            # Ws^T streamed in halves through the encT slots.
            dfb = const_pool.tile([P, MT, BPC], F32)
            for half in range(2):
                wst_sb = tp_pool.tile([P, KT, D // 2], BF16, tag="encT",
                                      name=f"wst_sb{half}")
                nc.sync.dma_start(
                    wst_sb[:],
                    wst_d[:, :, half * (D // 2):(half + 1) * (D // 2)]
                    .rearrange("k p e -> p k e"))
                for mm in range(MT // 2):
                    m = half * (MT // 2) + mm
                    ps_df = pt_psum.tile([P, BPC], F32, tag="pt",
                                         name=f"ps_df{m}")
                    for k in range(KT):
                        nc.tensor.matmul(
                            ps_df[:],
                            wst_sb[:, k, mm * P:(mm + 1) * P],
                            stt_sb[:, k, :],
                            start=(k == 0),
                            stop=(k == KT - 1),
                        )
                    nc.vector.tensor_scalar_add(
                        dfb[:, m, :], ps_df[:], bs_sb[:, m:m + 1])

            # Wh^T streamed per m-tile (the first EF chunk only needs m=0
            # to begin; later tiles arrive while earlier ones compute)
            wht_sb = const_pool.tile([P, MT, KT, P], BF16)
            for m in range(MT):
                nc.sync.dma_start(wht_sb[:, m, :, :],
                                  wht_d[m].rearrange("k p j -> p k j"))

            # ---------------- per-example stream ----------------
            stop_stage = os.environ.get("BASSK_STOP", "")
            n_ex = int(os.environ.get("BASSK_NB", BPC))
            for b in range(n_ex):
                # per-quarter chain: cast-load -> spill -> transpose-load.
                # Separate tiles per quarter so the scheduler can overlap
                # quarter q+1's DMA chain with quarter q's matmuls.
                enc_nat_qs = []
                encT_qs = []
                for q in range(4):
                    # 1) cast-load enc (fp32 -> bf16), natural [s, d]
                    enc_nat_q = nat_pool.tile([P, 4, D], BF16, tag="nat",
                                              name=f"enc_nat_{b}_{q}")
                    enc_nat_qs.append(enc_nat_q)
                    nc.gpsimd.dma_start(
                        enc_nat_q[:],
                        enc_d[b, 512 * q:512 * (q + 1), :].rearrange(
                            "(t p) d -> p t d", p=P),
                    )
                    # 2) TensorE transpose -> encT[d partition, s-chunk]
                    # (PE-side transposes keep the DMA rings free and the
                    # PE clock warm; identity is the stationary operand)
                    encT_q = tp_pool.tile([P, KT, CH], BF16, tag="encT",
                                          name=f"encT_{b}_{q}")
                    encT_qs.append(encT_q)
                    for m in range(KT):
                        ps_t = pt_psum.tile([P, CH], BF16, tag="pt",
                                            name=f"ps_t_{b}_{q}_{m}")
                        for t in range(4):
                            nc.tensor.transpose(
                                ps_t[:, t * P:(t + 1) * P],
                                enc_nat_q[:, t, m * P:(m + 1) * P],
                                id_sb[:])
                        nc.vector.tensor_copy(encT_q[:, m, :], ps_t[:])
                # mask row (exactly representable in bf16 for 0/1 masks)
                mask_t = rows_pool.tile([1, S], BF16, tag="mask")
                nc.gpsimd.dma_start(mask_t[:], mask_d[b:b + 1, :])

                if stop_stage == "dma":
                    # keep the DMA chain alive via a dummy output write
                    for q in range(4):
                        nc.gpsimd.dma_start(
                            oatt_d[b:b + 1, q * CH:(q + 1) * CH],
                            encT_qs[q][0:1, 0, :])
                    continue
                # 4) EF^T tiles + fused tanh + score matvec
                score_t = rows_pool.tile([1, S], F32, tag="score")
                for c in range(NCHUNK):
                    ps_sc = sc_psum.tile([1, CH], F32, tag="sc")
                    for m in range(MT):
                        ps_ef = ef_psum.tile([P, CH], F32, tag="ef")
                        for k in range(KT):
                            nc.tensor.matmul(
                                ps_ef[:],
                                wht_sb[:, m, k, :],
                                encT_qs[c][:, k, :],
                                start=(k == 0),
                                stop=(k == KT - 1),
                            )
                        t_t = t_pool.tile([P, CH], BF16, tag="tt")
                        nc.scalar.activation(
                            t_t[:], ps_ef[:], AF.Tanh,
                            bias=dfb[:, m, b:b + 1])
                        if stop_stage == "ef":
                            if m == 0:
                                nc.gpsimd.dma_start(
                                    oatt_d[b:b + 1, c * CH:(c + 1) * CH],
                                    t_t[0:1, :])
                            continue
                        nc.tensor.matmul(
                            ps_sc[:],
                            vv_sb[:, m:m + 1],
                            t_t[:],
                            start=(m == 0),
                            stop=(m == MT - 1),
                        )
                    if stop_stage == "ef":
                        continue
                    nc.vector.tensor_copy(
                        score_t[0:1, c * CH:(c + 1) * CH], ps_sc[:])
                if stop_stage == "ef":
                    continue
                if stop_stage == "score":
                    nc.sync.dma_start(oatt_d[b:b + 1, :], score_t[:])
                    continue

                # 5) masked softmax (no max-subtraction; |score| <= 32)
                e_t = rows_pool.tile([1, S], BF16, tag="e")
                nc.scalar.activation(e_t[:], score_t[:], AF.Exp)
                if stop_stage == "exp":
                    nc.gpsimd.dma_start(oatt_d[b:b + 1, :], e_t[:])
                    continue
                em_t = rows_pool.tile([1, S], F32, tag="em", bufs=1)
                sum_t = small_pool.tile([1, 1], F32, tag="sum")
                nc.vector.tensor_mul(em_t[:], e_t[:], mask_t[:])
                nc.vector.reduce_sum(sum_t[:], em_t[:],
                                     axis=mybir.AxisListType.X)
                if stop_stage == "ttr":
                    nc.sync.dma_start(oatt_d[b:b + 1, 0:1], sum_t[:])
                    nc.sync.dma_start(oatt_d[b:b + 1, 4:4 + S - 4],
                                      em_t[0:1, 0:S - 4])
                    continue
                recip_t = small_pool.tile([1, 1], F32, tag="recip")
                nc.vector.reciprocal(recip_t[:], sum_t[:])

                # attention-weight output row
                att_t = rows_pool.tile([1, S], BF16, tag="e", name="att_t")
                nc.vector.tensor_scalar_mul(att_t[:], em_t[:],
                                            recip_t[0:1, 0:1])
                nc.gpsimd.dma_start(oatt_d[b:b + 1, :], att_t[:])
                if stop_stage == "soft":
                    continue

                # 6) transpose unnormalized weights to partitions
                ps_wt = sc_psum.tile([P, ST], F32, tag="sc", name=f"ps_wt{b}")
                for j in range(ST):
                    nc.tensor.transpose(
                        ps_wt[:, j:j + 1],
                        em_t[0:1, j * P:(j + 1) * P],
                        ident[:])
                w_bf = small_pool.tile([P, ST], BF16, tag="wbf")
                nc.vector.tensor_copy(w_bf[:], ps_wt[:])
                if stop_stage == "wt":
                    nc.gpsimd.dma_start(octx_d[b:b + 1, 0:ST],
                                        w_bf[0:1, :])
                    continue

                # 7) context matvec over natural-layout enc
                ctx_t = rows_pool.tile([1, D], F32, tag="ctx", bufs=1)
                for hh in range(2):
                    ps_cx = cx_psum.tile([1, CH], F32, tag="cx")
                    for st in range(ST):
                        nc.tensor.matmul(
                            ps_cx[:],
                            w_bf[:, st:st + 1],
                            enc_nat_qs[st // 4][:, st % 4,
                                                hh * CH:(hh + 1) * CH],
                            start=(st == 0),
                            stop=(st == ST - 1),
                        )
                    nc.scalar.mul(ctx_t[0:1, hh * CH:(hh + 1) * CH],
                                  ps_cx[:], recip_t[0:1, 0:1])
                nc.scalar.dma_start(octx_d[b:b + 1, :], ctx_t[:])

    nc.compile()
    return nc


def _get_nc():
    if "nc" not in _CACHE:
        _CACHE["nc"] = _build_nc()
    return _CACHE["nc"]


def kernel(h_dec, c_dec, encoder_output, x_padding_masks, Wh, Ws, bs, v,
           _trace=False):
    global LAST_EXEC_NS, LAST_RESULTS
    h = np.asarray(h_dec, dtype=np.float32)
    c = np.asarray(c_dec, dtype=np.float32)
    enc = np.asarray(encoder_output, dtype=np.float32)
    mask = np.asarray(x_padding_masks, dtype=np.float32)
    Wh = np.asarray(Wh, dtype=np.float32)
    Ws = np.asarray(Ws, dtype=np.float32)
    bs = np.asarray(bs, dtype=np.float32)
    v = np.asarray(v, dtype=np.float32)

    # host-side prep of the small parameters only
    # wht[m, k, p, j] = Wh^T[k*128+p, m*128+j] (m-major tile order)
    wht = np.ascontiguousarray(
        Wh.T.reshape(KT, P, MT, P).transpose(2, 0, 1, 3)).astype(NPBF16)
    wst = np.ascontiguousarray(Ws.T).reshape(KT, P, D).astype(NPBF16)
    s_t = np.concatenate([h[0], c[0]], axis=1)          # [B, D]
    stt_full = np.ascontiguousarray(s_t.T)              # [D, B]
    vv = np.ascontiguousarray(v.reshape(KT, P).T).astype(NPBF16)
    bsr = np.ascontiguousarray(bs.reshape(KT, P).T).astype(np.float32)
    ident128 = np.eye(P, dtype=NPBF16)

    in_maps = []
    for i in range(NCORES):
        b0 = i * BPC
        stt_i = np.ascontiguousarray(
            stt_full[:, b0:b0 + BPC]).reshape(KT, P, BPC).astype(NPBF16)
        in_maps.append({
            "enc": np.ascontiguousarray(enc[b0:b0 + BPC]),
            "mask": np.ascontiguousarray(mask[b0:b0 + BPC]),
            "wht": wht,
            "wst": wst,
            "stt": stt_i,
            "vv": vv,
            "bsr": bsr,
            "ident128": ident128,
        })

    nc = _get_nc()
    res = run_bass_kernel_spmd(nc, in_maps, core_ids=list(range(NCORES)),
                               trace=_trace)
    LAST_EXEC_NS = res.exec_time_ns
    LAST_RESULTS = res
    ctx = np.concatenate([r["out_ctx"] for r in res.results], axis=0)
    att = np.concatenate([r["out_att"] for r in res.results], axis=0)
    return ctx, att


# revision 55
# speedup vs baseline: 1.0881x; 1.0635x over previous
"""Trainium2 Bass kernel for Bahdanau-style attention (nn_Attention).

  s_t   = concat(h_dec, c_dec)                      [B, D]
  EF    = enc @ Wh^T                                [B, S, D]
  df    = s_t @ Ws^T + bs                           [B, D]
  score = tanh(EF + df) @ v                         [B, S]
  w     = softmax(score) * mask, renormalized       [B, S]
  ctx   = w @ enc                                   [B, D]
  returns (ctx, w)

Sharding: data-parallel over batch B=32 across 8 cores (4 examples/core),
weights replicated. No collectives.

Per-core device plan (all heavy compute in bf16, fp32 accumulation):
  1. enc (fp32) is cast-loaded to bf16 via SWDGE DMA, stored to a DRAM
     scratch, and transpose-loaded back with the DMA xbar so the
     contraction dim d lands on partitions (encT[d, s]).
  2. EF^T[d_out, s] tiles = sum_k WhT[k,m].T @ encT[k, s] on TensorE.
     tanh(x + df) fused on ScalarE with per-partition bias, output bf16.
  3. score = v . T via M=1 matmuls accumulated over the 8 d_out tiles.
  4. softmax without max-subtraction (|score| <= sum|v| ~ 32, exp is safe
     in fp32 and mathematically identical to the max-subtracted form).
     mask apply + sum in one fused tensor_tensor_reduce.
  5. context = sum_s w[s] * enc[s, :] via M=1 matmuls over natural-layout
     bf16 enc tiles kept from step 1; normalized by 1/sum at the end.
"""

import os
import sys

for _p in ("/opt/trn_rl_repo", "/root/.axon_site/_ro/trn_rl_repo"):
    if os.path.isdir(_p) and _p not in sys.path:
        sys.path.insert(0, _p)

import ml_dtypes
import numpy as np

import concourse.bacc as bacc
import concourse.mybir as mybir
import concourse.tile as tile
from concourse.bass_utils import run_bass_kernel_spmd

BF16 = mybir.dt.bfloat16
F32 = mybir.dt.float32
NPBF16 = ml_dtypes.bfloat16

H = 512
D = 2 * H          # 1024
B = 32
S = 2048
NCORES = 8
BPC = B // NCORES  # 4 examples per core
P = 128            # partitions
KT = D // P        # 8 contraction tiles
MT = D // P        # 8 d_out tiles
ST = S // P        # 16 s tiles
CH = 512           # s-chunk width (max fp32 PSUM free dim)
NCHUNK = S // CH   # 4

AF = mybir.ActivationFunctionType
ALU = mybir.AluOpType

_CACHE = {}
LAST_EXEC_NS = None
LAST_RESULTS = None


def _build_nc():
    nc = bacc.Bacc("TRN2", target_bir_lowering=False, debug=False,
                   enable_asserts=False)

    enc_d = nc.dram_tensor("enc", [BPC, S, D], F32, kind="ExternalInput").ap()
    wht_d = nc.dram_tensor("wht", [MT, KT, P, P], BF16,
                           kind="ExternalInput").ap()
    wst_d = nc.dram_tensor("wst", [KT, P, D], BF16, kind="ExternalInput").ap()
    stt_d = nc.dram_tensor("stt", [KT, P, BPC], BF16, kind="ExternalInput").ap()
    vv_d = nc.dram_tensor("vv", [P, KT], BF16, kind="ExternalInput").ap()
    bsr_d = nc.dram_tensor("bsr", [P, KT], F32, kind="ExternalInput").ap()
    mask_d = nc.dram_tensor("mask", [BPC, S], F32, kind="ExternalInput").ap()
    id_d = nc.dram_tensor("ident128", [P, P], BF16, kind="ExternalInput").ap()
    octx_d = nc.dram_tensor("out_ctx", [BPC, D], F32, kind="ExternalOutput").ap()
    oatt_d = nc.dram_tensor("out_att", [BPC, S], F32, kind="ExternalOutput").ap()

    with tile.TileContext(nc) as tc:
        with (
            tc.tile_pool(name="const", bufs=1) as const_pool,
            tc.tile_pool(name="nat", bufs=8) as nat_pool,
            tc.tile_pool(name="tp", bufs=8) as tp_pool,
            tc.tile_pool(name="tt", bufs=3) as t_pool,
            tc.tile_pool(name="rows", bufs=2) as rows_pool,
            tc.tile_pool(name="smalls", bufs=2) as small_pool,
            tc.tile_pool(name="psum_ef", bufs=2, space="PSUM") as ef_psum,
            tc.tile_pool(name="psum_sc", bufs=2, space="PSUM") as sc_psum,
            tc.tile_pool(name="psum_cx", bufs=2, space="PSUM") as cx_psum,
            tc.tile_pool(name="psum_pt", bufs=2, space="PSUM") as pt_psum,
        ):
            # ---------------- one-time loads (sync HWDGE ring; the big
            # cast-loads are on the gpsimd SWDGE ring). Order matters:
            # small consts first so df + the first transposes can start
            # while the bigger weight tiles stream in. ------------------
            stt_sb = const_pool.tile([P, KT, BPC], BF16)
            nc.sync.dma_start(stt_sb[:], stt_d.rearrange("k p j -> p k j"))
            vv_sb = const_pool.tile([P, KT], BF16)
            nc.sync.dma_start(vv_sb[:], vv_d[:])
            bs_sb = const_pool.tile([P, KT], F32)
            nc.sync.dma_start(bs_sb[:], bsr_d[:])
            id_sb = const_pool.tile([P, P], BF16)
            nc.sync.dma_start(id_sb[:], id_d[:])
            ident = const_pool.tile([1, 1], F32)
            nc.vector.memset(ident[:], 1.0)

            # ---------------- decoder features df ----------------
            # dfb[d_out partition, m, b] = (Ws @ s_t^T)[d_out, b] + bs[d_out]
            # Ws^T streamed in halves through the encT slots.
            # ---------------- per-example stream ----------------
            # The bigger weight tensors (Ws, Wh) are loaded on the SAME
            # SWDGE FIFO as the enc cast-loads, interleaved after the
            # first casts: SWDGE completes strictly in order, so the
            # example-0 quarters get full bandwidth and arrival times
            # are deterministic.
            stop_stage = os.environ.get("BASSK_STOP", "")
            n_ex = int(os.environ.get("BASSK_NB", BPC))
            wst_sbs = []
            dfb = const_pool.tile([P, MT, BPC], F32)
            wht_sb = const_pool.tile([P, MT, KT, P], BF16)

            def load_wht(ms):
                for m in ms:
                    nc.gpsimd.dma_start(
                        wht_sb[:, m, :, :],
                        wht_d[m].rearrange("k p j -> p k j"))

            def load_wst(half):
                wst_sb = tp_pool.tile([P, KT, D // 2], BF16, tag="encT",
                                      name=f"wst_sb{half}")
                wst_sbs.append(wst_sb)
                nc.gpsimd.dma_start(
                    wst_sb[:],
                    wst_d[:, :, half * (D // 2):(half + 1) * (D // 2)]
                    .rearrange("k p e -> p k e"))

            def load_weights_piece(step):
                # issued between the early cast-loads, in FIFO order:
                # wht m-tiles arrive just ahead of the EF m-loop's needs
                if step == 0:
                    load_wht(range(0, 2))
                elif step == 1:
                    load_wht(range(2, 5))
                    load_wst(0)
                elif step == 2:
                    load_wht(range(5, MT))
                    load_wst(1)
                elif step == 3:
                    # decoder features, once the Ws halves are in
                    for half in range(2):
                        for mm in range(MT // 2):
                            m = half * (MT // 2) + mm
                            ps_df = pt_psum.tile([P, BPC], F32, tag="pt",
                                                 name=f"ps_df{m}")
                            for k in range(KT):
                                nc.tensor.matmul(
                                    ps_df[:],
                                    wst_sbs[half][:, k,
                                                  mm * P:(mm + 1) * P],
                                    stt_sb[:, k, :],
                                    start=(k == 0),
                                    stop=(k == KT - 1),
                                )
                            nc.vector.tensor_scalar_add(
                                dfb[:, m, :], ps_df[:], bs_sb[:, m:m + 1])
            for b in range(n_ex):
                # per-quarter chain: cast-load -> spill -> transpose-load.
                # Separate tiles per quarter so the scheduler can overlap
                # quarter q+1's DMA chain with quarter q's matmuls.
                enc_nat_qs = []
                encT_qs = []
                for q in range(4):
                    # 1) cast-load enc (fp32 -> bf16), natural [s, d]
                    enc_nat_q = nat_pool.tile([P, 4, D], BF16, tag="nat",
                                              name=f"enc_nat_{b}_{q}")
                    enc_nat_qs.append(enc_nat_q)
                    nc.gpsimd.dma_start(
                        enc_nat_q[:],
                        enc_d[b, 512 * q:512 * (q + 1), :].rearrange(
                            "(t p) d -> p t d", p=P),
                    )
                    if b == 0:
                        load_weights_piece(q)
                    # 2) TensorE transpose -> encT[d partition, s-chunk]
                    # (PE-side transposes keep the DMA rings free and the
                    # PE clock warm; identity is the stationary operand)
                    encT_q = tp_pool.tile([P, KT, CH], BF16, tag="encT",
                                          name=f"encT_{b}_{q}")
                    encT_qs.append(encT_q)
                    for m in range(KT):
                        ps_t = pt_psum.tile([P, CH], BF16, tag="pt",
                                            name=f"ps_t_{b}_{q}_{m}")
                        for t in range(4):
                            nc.tensor.transpose(
                                ps_t[:, t * P:(t + 1) * P],
                                enc_nat_q[:, t, m * P:(m + 1) * P],
                                id_sb[:])
                        nc.vector.tensor_copy(encT_q[:, m, :], ps_t[:])
                # mask row (exactly representable in bf16 for 0/1 masks)
                mask_t = rows_pool.tile([1, S], BF16, tag="mask")
                nc.gpsimd.dma_start(mask_t[:], mask_d[b:b + 1, :])

                if stop_stage == "dma":
                    # keep the DMA chain alive via a dummy output write
                    for q in range(4):
                        nc.gpsimd.dma_start(
                            oatt_d[b:b + 1, q * CH:(q + 1) * CH],
                            encT_qs[q][0:1, 0, :])
                    continue
                # 4) EF^T tiles + fused tanh + score matvec
                score_t = rows_pool.tile([1, S], F32, tag="score")
                for c in range(NCHUNK):
                    ps_sc = sc_psum.tile([1, CH], F32, tag="sc")
                    for m in range(MT):
                        ps_ef = ef_psum.tile([P, CH], F32, tag="ef")
                        for k in range(KT):
                            nc.tensor.matmul(
                                ps_ef[:],
                                wht_sb[:, m, k, :],
                                encT_qs[c][:, k, :],
                                start=(k == 0),
                                stop=(k == KT - 1),
                            )
                        t_t = t_pool.tile([P, CH], BF16, tag="tt")
                        nc.scalar.activation(
                            t_t[:], ps_ef[:], AF.Tanh,
                            bias=dfb[:, m, b:b + 1])
                        if stop_stage == "ef":
                            if m == 0:
                                nc.gpsimd.dma_start(
                                    oatt_d[b:b + 1, c * CH:(c + 1) * CH],
                                    t_t[0:1, :])
                            continue
                        nc.tensor.matmul(
                            ps_sc[:],
                            vv_sb[:, m:m + 1],
                            t_t[:],
                            start=(m == 0),
                            stop=(m == MT - 1),
                        )
                    if stop_stage == "ef":
                        continue
                    nc.vector.tensor_copy(
                        score_t[0:1, c * CH:(c + 1) * CH], ps_sc[:])
                if stop_stage == "ef":
                    continue
                if stop_stage == "score":
                    nc.sync.dma_start(oatt_d[b:b + 1, :], score_t[:])
                    continue

                # 5) masked softmax (no max-subtraction; |score| <= 32)
                e_t = rows_pool.tile([1, S], BF16, tag="e")
                nc.scalar.activation(e_t[:], score_t[:], AF.Exp)
                if stop_stage == "exp":
                    nc.gpsimd.dma_start(oatt_d[b:b + 1, :], e_t[:])
                    continue
                em_t = rows_pool.tile([1, S], F32, tag="em", bufs=1)
                sum_t = small_pool.tile([1, 1], F32, tag="sum")
                nc.vector.tensor_mul(em_t[:], e_t[:], mask_t[:])
                nc.vector.reduce_sum(sum_t[:], em_t[:],
                                     axis=mybir.AxisListType.X)
                if stop_stage == "ttr":
                    nc.sync.dma_start(oatt_d[b:b + 1, 0:1], sum_t[:])
                    nc.sync.dma_start(oatt_d[b:b + 1, 4:4 + S - 4],
                                      em_t[0:1, 0:S - 4])
                    continue
                recip_t = small_pool.tile([1, 1], F32, tag="recip")
                nc.vector.reciprocal(recip_t[:], sum_t[:])

                # attention-weight output row
                att_t = rows_pool.tile([1, S], BF16, tag="e", name="att_t")
                nc.vector.tensor_scalar_mul(att_t[:], em_t[:],
                                            recip_t[0:1, 0:1])
                nc.gpsimd.dma_start(oatt_d[b:b + 1, :], att_t[:])
                if stop_stage == "soft":
                    continue

                # 6) transpose unnormalized weights to partitions
                ps_wt = sc_psum.tile([P, ST], F32, tag="sc", name=f"ps_wt{b}")
                for j in range(ST):
                    nc.tensor.transpose(
                        ps_wt[:, j:j + 1],
                        em_t[0:1, j * P:(j + 1) * P],
                        ident[:])
                w_bf = small_pool.tile([P, ST], BF16, tag="wbf")
                nc.vector.tensor_copy(w_bf[:], ps_wt[:])
                if stop_stage == "wt":
                    nc.gpsimd.dma_start(octx_d[b:b + 1, 0:ST],
                                        w_bf[0:1, :])
                    continue

                # 7) context matvec over natural-layout enc
                ctx_t = rows_pool.tile([1, D], F32, tag="ctx", bufs=1)
                for hh in range(2):
                    ps_cx = cx_psum.tile([1, CH], F32, tag="cx")
                    for st in range(ST):
                        nc.tensor.matmul(
                            ps_cx[:],
                            w_bf[:, st:st + 1],
                            enc_nat_qs[st // 4][:, st % 4,
                                                hh * CH:(hh + 1) * CH],
                            start=(st == 0),
                            stop=(st == ST - 1),
                        )
                    nc.scalar.mul(ctx_t[0:1, hh * CH:(hh + 1) * CH],
                                  ps_cx[:], recip_t[0:1, 0:1])
                nc.scalar.dma_start(octx_d[b:b + 1, :], ctx_t[:])

    nc.compile()
    return nc


def _get_nc():
    if "nc" not in _CACHE:
        _CACHE["nc"] = _build_nc()
    return _CACHE["nc"]


def kernel(h_dec, c_dec, encoder_output, x_padding_masks, Wh, Ws, bs, v,
           _trace=False):
    global LAST_EXEC_NS, LAST_RESULTS
    h = np.asarray(h_dec, dtype=np.float32)
    c = np.asarray(c_dec, dtype=np.float32)
    enc = np.asarray(encoder_output, dtype=np.float32)
    mask = np.asarray(x_padding_masks, dtype=np.float32)
    Wh = np.asarray(Wh, dtype=np.float32)
    Ws = np.asarray(Ws, dtype=np.float32)
    bs = np.asarray(bs, dtype=np.float32)
    v = np.asarray(v, dtype=np.float32)

    # host-side prep of the small parameters only
    # wht[m, k, p, j] = Wh^T[k*128+p, m*128+j] (m-major tile order)
    wht = np.ascontiguousarray(
        Wh.T.reshape(KT, P, MT, P).transpose(2, 0, 1, 3)).astype(NPBF16)
    wst = np.ascontiguousarray(Ws.T).reshape(KT, P, D).astype(NPBF16)
    s_t = np.concatenate([h[0], c[0]], axis=1)          # [B, D]
    stt_full = np.ascontiguousarray(s_t.T)              # [D, B]
    vv = np.ascontiguousarray(v.reshape(KT, P).T).astype(NPBF16)
    bsr = np.ascontiguousarray(bs.reshape(KT, P).T).astype(np.float32)
    ident128 = np.eye(P, dtype=NPBF16)

    in_maps = []
    for i in range(NCORES):
        b0 = i * BPC
        stt_i = np.ascontiguousarray(
            stt_full[:, b0:b0 + BPC]).reshape(KT, P, BPC).astype(NPBF16)
        in_maps.append({
            "enc": np.ascontiguousarray(enc[b0:b0 + BPC]),
            "mask": np.ascontiguousarray(mask[b0:b0 + BPC]),
            "wht": wht,
            "wst": wst,
            "stt": stt_i,
            "vv": vv,
            "bsr": bsr,
            "ident128": ident128,
        })

    nc = _get_nc()
    res = run_bass_kernel_spmd(nc, in_maps, core_ids=list(range(NCORES)),
                               trace=_trace)
    LAST_EXEC_NS = res.exec_time_ns
    LAST_RESULTS = res
    ctx = np.concatenate([r["out_ctx"] for r in res.results], axis=0)
    att = np.concatenate([r["out_att"] for r in res.results], axis=0)
    return ctx, att
